# revision 1
# baseline (speedup 1.0000x reference)
"""DglGraphConvolution Trainium2 kernel — pure-matmul aggregation.

Per graph:
  1. PE: hidden = text @ W; kept in SBUF as bf16 [128, 32, 130]
     (32 windows of 128 node rows; col 128 = 1.0 degree lane, col 129 = 0).
  2. Edges sorted by (dst_window, src_window) into 32x32 blocks of the
     128x128 node grid; each block padded to exactly T_B=2 tiles of 128
     edge lanes (pad lanes have all-zero one-hot columns).
  3. For tile t (block b = t//2, ws = b % 32, wd = b // 32):
       mm1: gath_ps[128 lane, 130] = srcOH_t^T @ hidsb[:, ws, :]
            (lhsT = srcOH [128 src-node, 128 lane] bf16, shipped)
       copy: gath_sb bf16 <- gath_ps  (DVE/ACT)
       mm2: agg_ps[128 node, 130] += dstOH_t^T @ gath_sb
            (lhsT = dstOH [128 lane, 128 dst-node] bf16, shipped)
     agg_ps accumulates over the 64 tiles of each dst window; col 128 = deg.
  4. DVE: out = agg * 1/(deg+1) + bias per window.

Everything on device uses only plain DMA loads, matmuls, and elementwise
ops. Host-side work: sharding plus a bijective re-encoding of the edge
index lists into one-hot tiles (no arithmetic touches model data).
"""

import numpy as np

B, N, E, F = 16, 4096, 131072, 128
NCORES = 8
GPC = B // NCORES  # graphs per core
W = 128  # window size
NW = N // W  # 32
T_B = 2  # tiles per (wd, ws) block; Poisson(128) <= 256 w.p. ~1
T = NW * NW * T_B  # 2048 tiles per graph
HC = 130  # feature row: 128 | ones | pad
TPC = 64  # tiles per OH load chunk (= one dst window)
NCHUNK = T // TPC  # 32

_cache = {}


def _build_program():
    from contextlib import ExitStack

    import concourse.bacc as bacc
    import concourse.tile as tile
    from concourse import mybir
    from concourse._compat import get_trn_type
    from concourse.masks import make_identity

    f32 = mybir.dt.float32
    bf16 = mybir.dt.bfloat16

    nc = bacc.Bacc(get_trn_type() or "TRN2", target_bir_lowering=False, debug=False)

    text_d = nc.dram_tensor("text", [GPC, N, F], f32, kind="ExternalInput")
    w_d = nc.dram_tensor("weight", [F, F], f32, kind="ExternalInput")
    bias_d = nc.dram_tensor("biasrep", [128, F], f32, kind="ExternalInput")
    # pre-swizzled one-hots: [graph, chunk, lane/node, tile-in-chunk, 128]
    soh_d = nc.dram_tensor(
        "srcoh", [GPC, NCHUNK, 128, TPC, 128], bf16, kind="ExternalInput"
    )
    doh_d = nc.dram_tensor(
        "dstoh", [GPC, NCHUNK, 128, TPC, 128], bf16, kind="ExternalInput"
    )
    out_d = nc.dram_tensor("out", [GPC, N, F], f32, kind="ExternalOutput")

    with tile.TileContext(nc) as tc, ExitStack() as ctx:
        const = ctx.enter_context(tc.tile_pool(name="const", bufs=1))
        work = ctx.enter_context(tc.tile_pool(name="work", bufs=3))
        hpool = ctx.enter_context(tc.tile_pool(name="hpool", bufs=1))
        ohpool = ctx.enter_context(tc.tile_pool(name="ohp", bufs=2))
        gsb = ctx.enter_context(tc.tile_pool(name="gsb", bufs=4))
        psum = ctx.enter_context(tc.tile_pool(name="psum", bufs=1, space="PSUM"))
        gpsum = ctx.enter_context(tc.tile_pool(name="gpsum", bufs=3, space="PSUM"))
        apsum = ctx.enter_context(tc.tile_pool(name="apsum", bufs=1, space="PSUM"))

        ident = const.tile([128, 128], f32)
        make_identity(nc, ident[:])
        w_sb = const.tile([128, F], f32)
        nc.sync.dma_start(w_sb[:], w_d[:, :])
        bias_sb = const.tile([128, F], f32)
        nc.sync.dma_start(bias_sb[:], bias_d[:, :])

        agg_tiles = {}
        for g in range(GPC):
            # hidden = text @ W -> SBUF bf16 [128, 32, 130]
            hidsb = hpool.tile([128, NW, HC], bf16, tag="hidsb")
            nc.vector.memset(hidsb[:], 1.0)
            for c in range(NW):
                ttile = work.tile([128, F], f32, tag="text")
                nc.sync.dma_start(ttile[:], text_d[g, 128 * c : 128 * (c + 1), :])
                tT_ps = psum.tile([128, 128], f32, tag="tT")
                nc.tensor.transpose(out=tT_ps[:], in_=ttile[:], identity=ident[:])
                tT_sb = work.tile([128, 128], f32, tag="tTs")
                nc.vector.tensor_copy(tT_sb[:], tT_ps[:])
                h_ps = psum.tile([128, F], f32, tag="h")
                nc.tensor.matmul(
                    out=h_ps[:], lhsT=tT_sb[:], rhs=w_sb[:], start=True, stop=True
                )
                nc.scalar.activation(
                    hidsb[:, c, 0:F], h_ps[:], mybir.ActivationFunctionType.Copy
                )
                nc.vector.memset(hidsb[:, c, F + 1 : HC], 0.0)

            for chunk in range(NCHUNK):
                soh = ohpool.tile([128, TPC, 128], bf16, tag="soh")
                nc.sync.dma_start(soh[:], soh_d[g, chunk])
                doh = ohpool.tile([128, TPC, 128], bf16, tag="doh")
                nc.sync.dma_start(doh[:], doh_d[g, chunk])
                for tt in range(TPC):
                    t = chunk * TPC + tt
                    blk = t // T_B
                    ws = blk % NW
                    wd = blk // NW
                    j = t % TPC  # position within the dst window (== tt)
                    gath_ps = gpsum.tile([128, HC], f32, tag="gps")
                    nc.tensor.matmul(
                        out=gath_ps[:],
                        lhsT=soh[:, tt, :],
                        rhs=hidsb[:, ws, :],
                        start=True,
                        stop=True,
                    )
                    gath_sb = gsb.tile([128, HC], bf16, tag="gsb")
                    nc.vector.tensor_copy(gath_sb[:], gath_ps[:])
                    if j == 0:
                        agg_new = apsum.tile([128, HC], f32, tag=f"agg{wd % 2}")
                        agg_tiles[wd % 2] = agg_new
                    agg_ps = agg_tiles[wd % 2]
                    nc.tensor.matmul(
                        out=agg_ps[:],
                        lhsT=doh[:, tt, :],
                        rhs=gath_sb[:],
                        start=(j == 0),
                        stop=(j == TPC - 1),
                    )
                    if j == TPC - 1:
                        rec = work.tile([128, 1], f32, tag="rec")
                        nc.vector.tensor_scalar_add(
                            rec[:], agg_ps[:, F : F + 1], 1.0
                        )
                        nc.vector.reciprocal(rec[:], rec[:])
                        o1 = work.tile([128, F], f32, tag="o1")
                        nc.vector.tensor_tensor(
                            out=o1[:],
                            in0=agg_ps[:, 0:F],
                            in1=rec[:].to_broadcast([128, F]),
                            op=mybir.AluOpType.mult,
                        )
                        o2 = work.tile([128, F], f32, tag="o2")
                        nc.vector.tensor_add(o2[:], o1[:], bias_sb[:])
                        nc.sync.dma_start(
                            out_d[g, W * wd : W * (wd + 1), :], o2[:]
                        )

    nc.compile()
    return nc


def _prep_graph(src, dst):
    """(dst_window, src_window) block sort; returns one-hot packs
    soh, doh [NCHUNK, 128, TPC, 128] float32 (cast to bf16 by caller)."""
    ws = src // W
    wd = dst // W
    blk = wd * NW + ws
    order = np.argsort(blk, kind="stable")
    s, d, bo = src[order], dst[order], blk[order]
    counts = np.bincount(bo, minlength=NW * NW)
    assert counts.max() <= T_B * 128, f"block overflow: {counts.max()}"
    soh = np.zeros((T, 128, 128), dtype=np.float32)  # [tile, node, lane]
    doh = np.zeros((T, 128, 128), dtype=np.float32)  # [tile, lane, node]
    slo = (s % W).astype(np.int64)
    dlo = (d % W).astype(np.int64)
    starts = np.zeros(NW * NW + 1, dtype=np.int64)
    np.cumsum(counts, out=starts[1:])
    pos_in_blk = np.arange(len(s)) - starts[bo]
    tile_idx = bo * T_B + pos_in_blk // 128
    lane = pos_in_blk % 128
    soh[tile_idx, slo, lane] = 1.0
    doh[tile_idx, lane, dlo] = 1.0
    soh = soh.reshape(NCHUNK, TPC, 128, 128).transpose(0, 2, 1, 3).copy()
    doh = doh.reshape(NCHUNK, TPC, 128, 128).transpose(0, 2, 1, 3).copy()
    return soh, doh


def kernel(text, weight, bias, edge_src, edge_dst):
    import ml_dtypes

    text = np.asarray(text, dtype=np.float32)
    weight = np.asarray(weight, dtype=np.float32)
    bias = np.asarray(bias, dtype=np.float32)
    edge_src = np.asarray(edge_src, dtype=np.int32)
    edge_dst = np.asarray(edge_dst, dtype=np.int32)

    if "nc" not in _cache:
        _cache["nc"] = _build_program()
    nc = _cache["nc"]

    bias_rep = np.tile(bias[None, :], (128, 1)).astype(np.float32)

    in_maps = []
    for k in range(NCORES):
        soh = np.empty((GPC, NCHUNK, 128, TPC, 128), dtype=ml_dtypes.bfloat16)
        doh = np.empty((GPC, NCHUNK, 128, TPC, 128), dtype=ml_dtypes.bfloat16)
        for g in range(GPC):
            b = k * GPC + g
            so, do = _prep_graph(edge_src[b], edge_dst[b])
            soh[g] = so.astype(ml_dtypes.bfloat16)
            doh[g] = do.astype(ml_dtypes.bfloat16)
        in_maps.append(
            {
                "text": text[k * GPC : (k + 1) * GPC],
                "weight": weight,
                "biasrep": bias_rep,
                "srcoh": soh,
                "dstoh": doh,
            }
        )

    _cache["in_maps"] = in_maps

    from concourse.bass_utils import run_bass_kernel_spmd

    res = run_bass_kernel_spmd(nc, in_maps, list(range(NCORES)))
    out = np.concatenate([res.results[k]["out"] for k in range(NCORES)], axis=0)
    return out.astype(np.float32)



# revision 2
# speedup vs baseline: 5.5453x; 5.5453x over previous
"""DglGraphConvolution Trainium2 kernel — dense-adjacency matmul aggregation.

Per graph (4096 nodes, 131072 edges, F=128):
  1. PE: hidden[node, f] = text @ W via lhsT = textT tiles (text shipped
     pre-transposed [F, N], so no on-device transposes); cast bf16 to SBUF.
  2. Aggregation as one dense matmul against the adjacency-count matrix:
       aggT[f, dst] = sum_src hidden[src, f] * AT[src, dst]
     AT (bincount of edges, exact small ints) is shipped fp8_e4m3
     [chunk, 128, ws, 512]: per 512-dst chunk, 32 accumulating matmuls
     (lhsT = hidden window bf16 [128 src, 128 f], rhs = AT fp8 [128, 512])
     into one PSUM bank.
  3. Degree: deg+1 broadcast to all 128 partitions with a K=2 matmul
     (lhsT = ones [2,128], rhs = [deg; ones] slice), DVE reciprocal.
  4. Epilogue: out = aggT * recb (DVE) + bias[f] (ACT per-partition bias),
     DMA out as outT [F, N]; host un-transposes.

Host-side work is sharding plus re-encoding of the edge index lists into
adjacency counts / degree counts (np.bincount) — no arithmetic touches
model float data.
"""

import numpy as np

B, N, E, F = 16, 4096, 131072, 128
NCORES = 8
GPC = B // NCORES  # graphs per core
W = 128  # node window (matmul contraction tile)
NW = N // W  # 32
DC = 512  # dst columns per chunk (one PSUM bank of f32)
NCHUNK = N // DC  # 8

_cache = {}


def _build_program():
    from contextlib import ExitStack

    import concourse.bacc as bacc
    import concourse.tile as tile
    from concourse import mybir
    from concourse._compat import get_trn_type

    f32 = mybir.dt.float32
    bf16 = mybir.dt.bfloat16
    fp8 = mybir.dt.float8e4

    nc = bacc.Bacc(get_trn_type() or "TRN2", target_bir_lowering=False, debug=False)

    textT_d = nc.dram_tensor("textT", [GPC, F, N], f32, kind="ExternalInput")
    w_d = nc.dram_tensor("weight", [F, F], f32, kind="ExternalInput")
    bias_d = nc.dram_tensor("biascol", [F, 1], f32, kind="ExternalInput")
    # adjacency counts AT[src, dst] as [chunk, src_row, ws, dst_col]
    at_d = nc.dram_tensor("at8", [GPC, NCHUNK, W, NW, DC], fp8, kind="ExternalInput")
    # row 0: deg counts per dst node; row 1: ones
    deg_d = nc.dram_tensor("degrow", [GPC, 2, N], bf16, kind="ExternalInput")
    out_d = nc.dram_tensor("out", [GPC, F, N], f32, kind="ExternalOutput")

    with tile.TileContext(nc) as tc, ExitStack() as ctx:
        const = ctx.enter_context(tc.tile_pool(name="const", bufs=1))
        tpool = ctx.enter_context(tc.tile_pool(name="tpool", bufs=2))
        hpool = ctx.enter_context(tc.tile_pool(name="hpool", bufs=2))
        atpool = ctx.enter_context(tc.tile_pool(name="atp", bufs=3))
        rpool = ctx.enter_context(tc.tile_pool(name="rpool", bufs=2))
        dpool = ctx.enter_context(tc.tile_pool(name="dpool", bufs=2))
        opool = ctx.enter_context(tc.tile_pool(name="opool", bufs=4))
        hpsum = ctx.enter_context(tc.tile_pool(name="hpsum", bufs=2, space="PSUM"))
        dpsum = ctx.enter_context(tc.tile_pool(name="dpsum", bufs=2, space="PSUM"))
        apsum = ctx.enter_context(tc.tile_pool(name="apsum", bufs=2, space="PSUM"))

        w_sb = const.tile([F, F], f32)
        nc.sync.dma_start(w_sb[:], w_d[:, :])
        bias_sb = const.tile([F, 1], f32)
        nc.sync.dma_start(bias_sb[:], bias_d[:, :])
        ones2 = const.tile([2, F], bf16)
        nc.vector.memset(ones2[:], 1.0)

        for g in range(GPC):
            # ---- degree -> recb[f, dst] = 1/(deg[dst]+1) broadcast ----
            deg_sb = dpool.tile([2, N], bf16, tag="deg")
            nc.sync.dma_start(deg_sb[:], deg_d[g])
            recb = rpool.tile([F, N], f32, tag="recb")
            for c in range(NCHUNK):
                degb_ps = dpsum.tile([F, DC], f32, tag="degb")
                nc.tensor.matmul(
                    out=degb_ps[:],
                    lhsT=ones2[:],
                    rhs=deg_sb[:, DC * c : DC * (c + 1)],
                    start=True,
                    stop=True,
                )
                nc.vector.reciprocal(recb[:, DC * c : DC * (c + 1)], degb_ps[:])

            # ---- hidden[node, f] = text @ W, bf16 in SBUF ----
            textT_sb = tpool.tile([F, N], f32, tag="textT")
            nc.sync.dma_start(textT_sb[:], textT_d[g])
            hidsb = hpool.tile([W, NW, F], bf16, tag="hid")
            for c in range(NW):
                h_ps = hpsum.tile([W, F], f32, tag="h")
                nc.tensor.matmul(
                    out=h_ps[:],
                    lhsT=textT_sb[:, W * c : W * (c + 1)],
                    rhs=w_sb[:],
                    start=True,
                    stop=True,
                )
                nc.scalar.activation(
                    hidsb[:, c, :], h_ps[:], mybir.ActivationFunctionType.Copy
                )

            # ---- aggT[f, dst] += hid[ws].T @ AT[ws, chunk] ----
            for c in range(NCHUNK):
                at_sb = atpool.tile([W, NW, DC], fp8, tag="at")
                nc.sync.dma_start(at_sb[:], at_d[g, c])
                agg_ps = apsum.tile([F, DC], f32, tag="agg")
                for ws in range(NW):
                    nc.tensor.matmul(
                        out=agg_ps[:],
                        lhsT=hidsb[:, ws, :],
                        rhs=at_sb[:, ws, :],
                        start=(ws == 0),
                        stop=(ws == NW - 1),
                    )
                o_sb = opool.tile([F, DC], f32, tag="o")
                nc.vector.tensor_tensor(
                    out=o_sb[:],
                    in0=agg_ps[:],
                    in1=recb[:, DC * c : DC * (c + 1)],
                    op=mybir.AluOpType.mult,
                )
                o2_sb = opool.tile([F, DC], f32, tag="o2")
                nc.scalar.activation(
                    o2_sb[:],
                    o_sb[:],
                    mybir.ActivationFunctionType.Identity,
                    bias=bias_sb[:, 0:1],
                )
                nc.sync.dma_start(out_d[g, :, DC * c : DC * (c + 1)], o2_sb[:])

    nc.compile()
    return nc


def kernel(text, weight, bias, edge_src, edge_dst):
    import ml_dtypes

    text = np.asarray(text, dtype=np.float32)
    weight = np.asarray(weight, dtype=np.float32)
    bias = np.asarray(bias, dtype=np.float32)
    edge_src = np.asarray(edge_src, dtype=np.int64)
    edge_dst = np.asarray(edge_dst, dtype=np.int64)

    if "nc" not in _cache:
        _cache["nc"] = _build_program()
    nc = _cache["nc"]

    in_maps = []
    for k in range(NCORES):
        at8 = np.empty((GPC, NCHUNK, W, NW, DC), dtype=ml_dtypes.float8_e4m3)
        degrow = np.empty((GPC, 2, N), dtype=ml_dtypes.bfloat16)
        textT = np.empty((GPC, F, N), dtype=np.float32)
        for g in range(GPC):
            b = k * GPC + g
            src, dst = edge_src[b], edge_dst[b]
            cnt = np.bincount(src * N + dst, minlength=N * N)
            assert cnt.max() <= 15, f"edge multiplicity overflow: {cnt.max()}"
            # AT[src, dst] -> [chunk, src_row, ws, dst_col]
            at = cnt.astype(np.float32).reshape(NW, W, NCHUNK, DC)
            at8[g] = at.transpose(2, 1, 0, 3).astype(ml_dtypes.float8_e4m3)
            degrow[g, 0] = np.bincount(dst, minlength=N).astype(ml_dtypes.bfloat16)
            degrow[g, 1] = 1.0
            textT[g] = text[b].T
        in_maps.append(
            {
                "textT": textT,
                "weight": weight,
                "biascol": bias[:, None].copy(),
                "at8": at8,
                "degrow": degrow,
            }
        )

    _cache["in_maps"] = in_maps

    from concourse.bass_utils import run_bass_kernel_spmd

    res = run_bass_kernel_spmd(nc, in_maps, list(range(NCORES)))
    out = np.concatenate(
        [res.results[k]["out"].transpose(0, 2, 1) for k in range(NCORES)], axis=0
    )
    return np.ascontiguousarray(out).astype(np.float32)


# revision 4
# speedup vs baseline: 6.6034x; 1.1908x over previous
"""DglGraphConvolution Trainium2 kernel — dense-adjacency matmul aggregation.

out = (A @ text) @ W / (deg+1) + bias, per graph (N=4096 nodes, F=128).

Per graph:
  1. text [N, F] f32 DMA'd, cast bf16 on DVE -> textb [128, ws, 128]
     (partition = node % 128 within window ws).
  2. Aggregation vs the dense adjacency-count matrix AT[src, dst]
     (bincount of edges, exact small ints, shipped fp8_e4m3):
     per 512-dst chunk, 32 accumulating matmuls
       aggT[fin, dst] += textb[:, ws, :].T @ AT[ws, chunk]  (bf16 x fp8)
     into one PSUM bank; evacuated to SBUF as bf16 (ACT).
  3. W-apply: per 128-dst window, out_ps[dst, f] = aggTb[:, w].T @ Wb
     -- output lands in natural [node, f] orientation.
  4. Epilogue: out = out_ps * rec[:, w] (DVE, free-broadcast [128,1])
     + bias_rep (ACT); rec = 1/(deg+1) from shipped bincount counts,
     computed once per graph on a [128, NW] tile.
  5. DMA out in [128(part), w, f] layout; host un-shuffles windows.

Host-side work is sharding plus re-encoding of the edge index lists into
adjacency counts / degree counts (np.bincount) and layout shuffles — no
arithmetic touches model float data.
"""

import numpy as np

B, N, E, F = 16, 4096, 131072, 128
NCORES = 8
GPC = B // NCORES  # graphs per core
W = 128  # node window (matmul contraction tile)
NW = N // W  # 32
DC = 512  # dst columns per chunk (one PSUM bank of f32)
NCHUNK = N // DC  # 8
WPC = DC // W  # windows per chunk = 4
PAIR = 2  # chunks per AT DMA transfer (4 MB)
NPAIR = NCHUNK // PAIR  # 4

_cache = {}


def _build_program():
    from contextlib import ExitStack

    import concourse.bacc as bacc
    import concourse.tile as tile
    from concourse import mybir
    from concourse._compat import get_trn_type

    f32 = mybir.dt.float32
    bf16 = mybir.dt.bfloat16
    fp8 = mybir.dt.float8e4

    nc = bacc.Bacc(get_trn_type() or "TRN2", target_bir_lowering=False, debug=False)

    # text in window layout: [g, p, ws, f] = text[g, ws*128+p, f]
    text_d = nc.dram_tensor("textw", [GPC, W, NW, F], f32, kind="ExternalInput")
    w_d = nc.dram_tensor("weight", [F, F], f32, kind="ExternalInput")
    bias_d = nc.dram_tensor("biasrep", [W, F], f32, kind="ExternalInput")
    # adjacency counts AT[src, dst] as [pair, src_row, half, ws, dst_col]
    at_d = nc.dram_tensor(
        "at8", [GPC, NPAIR, W, PAIR, NW, DC], fp8, kind="ExternalInput"
    )
    # degree counts in window layout [p, w] = deg[w*128+p]
    deg_d = nc.dram_tensor("degw", [GPC, W, NW], f32, kind="ExternalInput")
    # out in window layout: [g, p, w, f] = out[g, w*128+p, f]
    out_d = nc.dram_tensor("out", [GPC, W, NW, F], f32, kind="ExternalOutput")

    with tile.TileContext(nc) as tc, ExitStack() as ctx:
        const = ctx.enter_context(tc.tile_pool(name="const", bufs=1))
        tfpool = ctx.enter_context(tc.tile_pool(name="tf", bufs=2))
        tbpool = ctx.enter_context(tc.tile_pool(name="tb", bufs=2))
        atpool = ctx.enter_context(tc.tile_pool(name="atp", bufs=3))
        gpool = ctx.enter_context(tc.tile_pool(name="gp", bufs=3))
        rpool = ctx.enter_context(tc.tile_pool(name="rp", bufs=2))
        opool = ctx.enter_context(tc.tile_pool(name="op", bufs=3))
        apsum = ctx.enter_context(tc.tile_pool(name="apsum", bufs=2, space="PSUM"))
        opsum = ctx.enter_context(tc.tile_pool(name="opsum", bufs=4, space="PSUM"))

        w_sb = const.tile([F, F], f32)
        nc.sync.dma_start(w_sb[:], w_d[:, :])
        w_bf = const.tile([F, F], bf16)
        nc.vector.tensor_copy(w_bf[:], w_sb[:])
        bias_sb = const.tile([W, F], f32)
        nc.sync.dma_start(bias_sb[:], bias_d[:, :])

        for g in range(GPC):
            # rec[p, w] = 1/(deg+1)
            deg_sb = rpool.tile([W, NW], f32, tag="deg")
            nc.sync.dma_start(deg_sb[:], deg_d[g])
            rec = rpool.tile([W, NW], f32, tag="rec")
            nc.vector.tensor_scalar_add(rec[:], deg_sb[:], 1.0)
            nc.vector.reciprocal(rec[:], rec[:])

            # text -> bf16 windows
            textf = tfpool.tile([W, NW, F], f32, tag="tf")
            nc.sync.dma_start(textf[:], text_d[g])
            textb = tbpool.tile([W, NW, F], bf16, tag="tb")
            nc.vector.tensor_copy(textb[:], textf[:])

            # aggregation + W-apply, pipelined: W-apply for chunk c is
            # emitted between agg chunks c+1 and c+2 on the PE queue.
            pend = []  # (agg_sb_bf16, chunk_idx)

            def wapply(entry):
                aggb, c = entry
                o_acc = opool.tile([W, WPC, F], f32, tag="oacc")
                for q in range(WPC):
                    w = c * WPC + q
                    out_ps = opsum.tile([W, F], f32, tag="ops")
                    nc.tensor.matmul(
                        out=out_ps[:],
                        lhsT=aggb[:, W * q : W * (q + 1)],
                        rhs=w_bf[:],
                        start=True,
                        stop=True,
                    )
                    nc.scalar.activation(
                        o_acc[:, q, :],
                        out_ps[:],
                        mybir.ActivationFunctionType.Identity,
                        bias=0.0,
                        scale=rec[:, w : w + 1],
                    )
                    nc.vector.tensor_add(o_acc[:, q, :], o_acc[:, q, :], bias_sb[:])
                nc.sync.dma_start(out_d[g, :, WPC * c : WPC * (c + 1), :], o_acc[:])

            for pr in range(NPAIR):
                at_sb = atpool.tile([W, PAIR, NW, DC], fp8, tag="at")
                nc.sync.dma_start(at_sb[:], at_d[g, pr])
                for h in range(PAIR):
                    c = pr * PAIR + h
                    agg_ps = apsum.tile([F, DC], f32, tag="agg")
                    for ws in range(NW):
                        nc.tensor.matmul(
                            out=agg_ps[:],
                            lhsT=textb[:, ws, :],
                            rhs=at_sb[:, h, ws, :],
                            start=(ws == 0),
                            stop=(ws == NW - 1),
                        )
                    aggb = gpool.tile([F, DC], bf16, tag="aggb")
                    nc.scalar.activation(
                        aggb[:], agg_ps[:], mybir.ActivationFunctionType.Copy
                    )
                    pend.append((aggb, c))
                    if len(pend) > 1:
                        wapply(pend.pop(0))
            while pend:
                wapply(pend.pop(0))

    nc.compile()
    return nc


def kernel(text, weight, bias, edge_src, edge_dst):
    import ml_dtypes

    text = np.asarray(text, dtype=np.float32)
    weight = np.asarray(weight, dtype=np.float32)
    bias = np.asarray(bias, dtype=np.float32)
    edge_src = np.asarray(edge_src, dtype=np.int64)
    edge_dst = np.asarray(edge_dst, dtype=np.int64)

    if "nc" not in _cache:
        _cache["nc"] = _build_program()
    nc = _cache["nc"]

    bias_rep = np.tile(bias[None, :], (W, 1)).astype(np.float32)

    in_maps = []
    for k in range(NCORES):
        at8 = np.empty((GPC, NPAIR, W, PAIR, NW, DC), dtype=ml_dtypes.float8_e4m3)
        degw = np.empty((GPC, W, NW), dtype=np.float32)
        textw = np.empty((GPC, W, NW, F), dtype=np.float32)
        for g in range(GPC):
            b = k * GPC + g
            src, dst = edge_src[b], edge_dst[b]
            cnt = np.bincount(src * N + dst, minlength=N * N)
            assert cnt.max() <= 15, f"edge multiplicity overflow: {cnt.max()}"
            # AT[src, dst] -> [pair, src_row, half, ws, dst_col]
            at = cnt.astype(np.float32).reshape(NW, W, NPAIR, PAIR, DC)
            at8[g] = at.transpose(2, 1, 3, 0, 4).astype(ml_dtypes.float8_e4m3)
            degw[g] = (
                np.bincount(dst, minlength=N).astype(np.float32).reshape(NW, W).T
            )
            textw[g] = text[b].reshape(NW, W, F).transpose(1, 0, 2)
        in_maps.append(
            {
                "textw": textw,
                "weight": weight,
                "biasrep": bias_rep,
                "at8": at8,
                "degw": degw,
            }
        )

    _cache["in_maps"] = in_maps

    from concourse.bass_utils import run_bass_kernel_spmd

    res = run_bass_kernel_spmd(nc, in_maps, list(range(NCORES)))
    # res out: [GPC, 128, NW, F] window layout -> [GPC, N, F]
    out = np.concatenate(
        [
            res.results[k]["out"].transpose(0, 2, 1, 3).reshape(GPC, N, F)
            for k in range(NCORES)
        ],
        axis=0,
    )
    return np.ascontiguousarray(out).astype(np.float32)
